# revision 5
# baseline (speedup 1.0000x reference)
"""DebertaV2 disentangled attention, 8 TRN2 cores (Bass/Tile), v2.

Head-sharded TP (2 heads/core). Single rel load (pk projection written
column-reversed during PSUM evacuation), host-packed weights, batched skew
bounces, AllToAll for the output resharding, per-core LayerNorm on 128 rows.
"""

import math

import numpy as np

H = 16
D = 64
HID = 1024
N = 1024
K = 1024
EPS = 1e-7
NCORES = 8
HPC = H // NCORES   # heads per core = 2
DPC = HPC * D       # head dims per core = 128
SCALE = 1.0 / math.sqrt(3.0 * D)

W_WIN = 1151        # skew window width (127 + 1024)
P = 128
FP8_SKEW = True

_CACHE = {}


def _build():
    import concourse.bass as bass
    import concourse.mybir as mybir
    import concourse.tile as tile
    from concourse import bacc
    from concourse.masks import make_identity
    from contextlib import ExitStack

    f32 = mybir.dt.float32
    bf16 = mybir.dt.bfloat16
    skew_dt = mybir.dt.float8e4 if FP8_SKEW else bf16

    nc = bacc.Bacc(None, target_bir_lowering=False, debug=False)
    names = {}

    with tile.TileContext(nc) as tc, ExitStack() as es:
        dio = es.enter_context(tc.tile_pool(name="dram_io", bufs=1, space="DRAM"))
        dwork = es.enter_context(tc.tile_pool(name="dram_work", bufs=1, space="DRAM"))

        def din(nm, shape, dt=bf16):
            t = dio.tile(shape, dt, kind="ExternalInput", name=nm, tag=nm)
            names[nm] = t.name
            return t

        hst = din("hst", (P, 8 * N))          # hs.T packed: [p, kt*1024+c]
        relT = din("relT", (P, 8 * 2 * K), mybir.dt.float8e4)  # rel.T fp8
        wpack = din("wpack", (P, 5 * 8 * P))  # q,k,v,pk,pq kt-blocks (bf16)
        wpos8 = din("wpos8", (P, 2 * 8 * P), mybir.dt.float8e4)  # pk,pq fp8
        wot = din("wot", (DPC, HID))          # Wo.T slice for this core
        hsr = din("hsr", (P, HID), f32)       # hs rows + bo (host-folded)
        bias8 = din("bias8", (P, 8), f32)     # cols: bq,bk,bv,bpk,bpq
        gl = din("gl", (1, 2 * HID), f32)     # [ln_g | ln_b]

        out_t = dio.tile((P, HID), f32, kind="ExternalOutput", name="out", tag="out")
        names["out"] = out_t.name

        # ---- SBUF pools --------------------------------------------------
        wt = es.enter_context(tc.tile_pool(name="wt", bufs=1))
        work = es.enter_context(tc.tile_pool(name="work", bufs=1))
        psC = es.enter_context(tc.tile_pool(name="psC", bufs=2, space="PSUM"))
        psS = es.enter_context(tc.tile_pool(name="psS", bufs=2, space="PSUM"))
        psB = es.enter_context(tc.tile_pool(name="psB", bufs=1, space="PSUM"))

        Iden = mybir.ActivationFunctionType.Identity
        Exp = mybir.ActivationFunctionType.Exp
        Sqrt = mybir.ActivationFunctionType.Sqrt
        ADD = mybir.AluOpType.add
        MUL = mybir.AluOpType.mult
        SUB = mybir.AluOpType.subtract
        BYP = mybir.AluOpType.bypass

        # ---- persistent inputs in SBUF ----------------------------------
        relT_sb, relT_free = tc.tile([P, 8 * 2 * K], mybir.dt.float8e4,
                                     name="relT_sb")
        hst_sb, hst_free = tc.tile([P, 8 * N], bf16, name="hst_sb")
        wp_sb = wt.tile([P, 5 * 8 * P], bf16, name="wp_sb", tag="wp_sb")
        nc.sync.dma_start(wp_sb[:, 0:16 * P], wpack[:, 0:16 * P])
        b8_sb = wt.tile([P, 8], f32, name="b8_sb", tag="b8_sb")
        nc.sync.dma_start(b8_sb[:], bias8[:])
        nc.sync.dma_start(hst_sb[:, 0:4 * N], hst[:, 0:4 * N])
        nc.sync.dma_start(hst_sb[:, 4 * N:8 * N], hst[:, 4 * N:8 * N])
        nc.sync.dma_start(wp_sb[:, 16 * P:40 * P], wpack[:, 16 * P:40 * P])
        w8_sb = wt.tile([P, 2 * 8 * P], mybir.dt.float8e4, name="w8_sb", tag="w8_sb")
        nc.sync.dma_start(w8_sb[:], wpos8[:])
        nc.sync.dma_start(relT_sb[:, 0:8 * K], relT[:, 0:8 * K])
        nc.sync.dma_start(relT_sb[:, 8 * K:16 * K], relT[:, 8 * K:16 * K])
        gl_sb = wt.tile([1, 2 * HID], f32, name="gl_sb", tag="gl_sb")
        nc.sync.dma_start(gl_sb[:], gl[:])
        hsr_sb = wt.tile([P, HID], f32, name="hsr_sb", tag="hsr_sb")
        nc.sync.dma_start(hsr_sb[:], hsr[:])

        def wsl(kind, kt):
            base = (kind * 8 + kt) * P
            return wp_sb[:, base:base + P]

        def bcol(k):
            return b8_sb[:, k:k + 1]

        def wsl8(kind, kt):
            base = (kind * 8 + kt) * P
            return w8_sb[:, base:base + P]

        ident = wt.tile([P, P], skew_dt, name="ident", tag="ident")
        make_identity(nc, ident[:])

        ones1 = wt.tile([1, P], bf16, name="ones1", tag="ones1")
        nc.vector.memset(ones1[:], 1.0)
        dume = wt.tile([1, 2], f32, name="dume", tag="dume")
        nc.vector.memset(dume[:], 1.0)
        nc.scalar.activation(dume[:, 0:1], dume[:, 1:2], Exp)
        gl_bf = wt.tile([1, 2 * HID], bf16, name="gl_bf", tag="gl_bf")
        nc.vector.tensor_copy(gl_bf[:], gl_sb[:])

        # ---- projections -------------------------------------------------
        # qT/kT: [128 (2 heads x 64 d), 1024 seq]
        qT = wt.tile([P, N], bf16, name="qT", tag="qT")
        kT = wt.tile([P, N], bf16, name="kT", tag="kT")

        def project_1024(dst, kind, rhs_sb, rhs_block, bias, rev=False):
            """dst[:, :1024] = sum_kt wsl(kind,kt).T @ rhs[kt block cols]."""
            ps = psS.tile([P, N], f32, name="pp", tag="st2")
            for c in range(2):
                for kt in range(8):
                    nc.tensor.matmul(ps[:, 512 * c:512 * (c + 1)], wsl(kind, kt),
                                     rhs_sb[:, rhs_block * 8192 + kt * N + 512 * c:
                                            rhs_block * 8192 + kt * N + 512 * (c + 1)],
                                     start=(kt == 0), stop=(kt == 7))
            if rev:
                ap = dst[:]
                rev_ap = bass.AP(ap.tensor, ap.offset + (N - 1),
                                 [[ap.ap[0][0], P], [-1, N]])
                nc.scalar.activation(rev_ap, ps[:], Iden, bias=bias)
            else:
                nc.scalar.activation(dst[:, 0:N], ps[:], Iden, bias=bias)

        project_1024(qT, 0, hst_sb, 0, bcol(0))
        project_1024(kT, 1, hst_sb, 0, bcol(1))

        # pos projections: pkT written column-REVERSED (== posk[2047-c]),
        # pqT normal. Both from the single normal relT.
        pkT = wt.tile([P, 2 * K], bf16, name="pkT", tag="pkT")
        pqT = wt.tile([P, 2 * K], bf16, name="pqT", tag="pqT")

        DR = mybir.MatmulPerfMode.DoubleRow

        def project_pos(dst, kind, bias, rev):
            wap = w8_sb[:]
            rap = relT_sb[:]
            for half in range(2):
                ps = psS.tile([P, N], f32, name="pq", tag="st2")
                for c in range(2):
                    for kj in range(4):  # pairs of kt blocks, K=256 each
                        lhs = bass.AP(
                            wap.tensor,
                            wap.offset + (kind * 8 + 2 * kj) * P,
                            [[wap.ap[0][0], P], [P, 2], [1, P]])
                        col = (2 * kj) * 2 * K + half * N + 512 * c
                        rhs = bass.AP(
                            rap.tensor, rap.offset + col,
                            [[rap.ap[0][0], P], [2 * K, 2], [1, 512]])
                        nc.tensor.matmul(ps[:, 512 * c:512 * (c + 1)], lhs, rhs,
                                         start=(kj == 0), stop=(kj == 3),
                                         perf_mode=DR)
                if rev:
                    # psum col c (global half*N + c) -> dst col 2047 - (half*N+c)
                    ap = dst[:]
                    base = ap.offset + (2 * K - 1 - half * N)
                    rev_ap = bass.AP(ap.tensor, base, [[ap.ap[0][0], P], [-1, N]])
                    nc.scalar.activation(rev_ap, ps[:], Iden, bias=bias)
                else:
                    nc.scalar.activation(dst[:, half * N:(half + 1) * N], ps[:],
                                         Iden, bias=bias)

        project_pos(pkT, 0, bcol(3), rev=True)
        project_pos(pqT, 1, bcol(4), rev=False)

        # va[jt]: [128 j, 132] = [v_h0(64) | one | pad | v_h1(64) | one | pad]
        # (projected after pk/pq so PE fills the c2p bounce window)
        va = []
        for jt in range(8):
            t = wt.tile([P, 132], bf16, name=f"va{jt}", tag=f"va{jt}")
            ps = psC.tile([P, 512], f32, name="pv", tag="pchunk")
            for kt in range(8):
                nc.tensor.matmul(ps[:, 0:DPC],
                                 hst_sb[:, kt * N + P * jt:kt * N + P * (jt + 1)],
                                 wsl(2, kt), start=(kt == 0), stop=(kt == 7))
            nc.scalar.copy(t[:, 0:64], ps[:, 0:64])
            nc.scalar.copy(t[:, 66:130], ps[:, 64:128])
            nc.vector.memset(t[:, 64:65], 1.0)
            nc.vector.memset(t[:, 130:131], 1.0)
            va.append(t)
        hst_free()
        relT_free()

        # ---- skew helper -------------------------------------------------
        evac_i = [0]

        def evac(dst_ap, src_ap):
            if evac_i[0] % 2 == 0:
                nc.vector.tensor_copy(dst_ap, src_ap)
            else:
                nc.scalar.copy(dst_ap, src_ap)
            evac_i[0] += 1

        def skew_group(lhsT, srcT, blocks, nm, gtag, gbufs, wide=False):
            """blocks: list of (lhs_col0, hd_slice, w0). Returns gathered tile
            [128, len(blocks)*1024]: g[:, B*1024+x] = blk_B[p, 127-p+x].
            wide=True: first 1024 cols go through a 2-bank psS tile with a
            single wide evacuation (c2p phase only — psS is idle there)."""
            nb = len(blocks)
            blk = work.tile([P, nb * W_WIN], skew_dt, name=f"blk_{nm}",
                            tag=f"blk{nb}", bufs=2)
            for B, (c0l, hd, w0) in enumerate(blocks):
                if wide:
                    psw = psS.tile([P, N], f32, name="pblkw", tag="st2")
                    for c0 in (0, 512):
                        nc.tensor.matmul(psw[:, c0:c0 + 512],
                                         lhsT[hd, c0l:c0l + P],
                                         srcT[hd, w0 + c0:w0 + c0 + 512],
                                         start=True, stop=True)
                    evac(blk[:, B * W_WIN:B * W_WIN + N], psw[:])
                    tails = ((1024, 127),)
                else:
                    tails = ((0, 512), (512, 512), (1024, 127))
                for (c0, w) in tails:
                    ps = psC.tile([P, 512], f32, name="pblk", tag="pchunk")
                    nc.tensor.matmul(ps[:, 0:w],
                                     lhsT[hd, c0l:c0l + P],
                                     srcT[hd, w0 + c0:w0 + c0 + w],
                                     start=True, stop=True)
                    evac(blk[:, B * W_WIN + c0:B * W_WIN + c0 + w], ps[:, 0:w])
            scr = dwork.tile((P * nb * W_WIN,), skew_dt, name=f"scr_{nm}",
                             tag=f"scr{nb}", bufs=2)
            h = scr[:].tensor
            g = work.tile([P, nb * N], skew_dt, name=f"g_{nm}", tag=gtag,
                          bufs=gbufs)
            for sub in range(0, nb, 2):
                nc.sync.dma_start(
                    bass.AP(h, scr[:].offset + sub * W_WIN,
                            [[nb * W_WIN, P], [1, 2 * W_WIN]]),
                    blk[:, sub * W_WIN:(sub + 2) * W_WIN])
                nc.sync.dma_start(
                    g[:, sub * N:(sub + 2) * N],
                    bass.AP(h, scr[:].offset + 127 + sub * W_WIN,
                            [[nb * W_WIN - 1, P], [W_WIN, 2], [1, N]]))
            return g

        # ---- c2p gather: 4 groups of (2 i-tiles x 2 heads) ---------------
        # block (r, h): lhsT=qT[hd, 128r:...], src=pkT(rev), w0 = 896-128r
        g_c = []
        for grp in range(4):
            blocks = []
            for dr in range(2):
                r = 2 * grp + dr
                for h in range(HPC):
                    hd = slice(64 * h, 64 * h + 64)
                    blocks.append((P * r, hd, 896 - 128 * r))
            g_c.append(skew_group(qT, pkT, blocks, f"c{grp}", f"g_c{grp}", 1,
                                  wide=True))

        # ---- per-head attention -----------------------------------------
        ctxT = wt.tile([P, N], bf16, name="ctxT", tag="ctxT")

        # hoist head0's first two p2c groups: their bounces overlap c2p's
        g_p0 = {}
        for gp in range(2):
            hd0 = slice(0, 64)
            blocks = [(P * (2 * gp + dj), hd0, 897 - 128 * (2 * gp + dj))
                      for dj in range(2)]
            g_p0[gp] = skew_group(kT, pqT, blocks, f"p0_{gp}", "g_p", 2)

        def c2p_slice(r, h, jt):
            g = g_c[r // 2]
            B = 2 * (r % 2) + h
            return g[:, B * N + P * jt:B * N + P * (jt + 1)]

        for h in range(HPC):
            hd = slice(64 * h, 64 * h + 64)
            pb = psB.tile([65, N], f32, name="pb", tag="pb")
            g_p = dict(g_p0) if h == 0 else {}
            for jt in range(8):
                if jt % 2 == 0 and jt // 2 not in g_p:
                    blocks = [(P * (jt + dj), hd, 897 - 128 * (jt + dj))
                              for dj in range(2)]
                    g_p[jt // 2] = skew_group(kT, pqT, blocks,
                                              f"p{h}_{jt // 2}", "g_p", 2)
                p2cT = g_p[jt // 2]
                pB = jt % 2
                st = psS.tile([P, N], f32, name="st", tag="st2")
                for c in range(2):
                    nc.tensor.matmul(st[:, 512 * c:512 * (c + 1)],
                                     kT[hd, P * jt:P * (jt + 1)],
                                     qT[hd, 512 * c:512 * (c + 1)],
                                     start=True, stop=False)
                    for rr in range(4):
                        r = 4 * c + rr
                        nc.tensor.matmul(st[:, 512 * c + P * rr:512 * c + P * (rr + 1)],
                                         c2p_slice(r, h, jt), ident[:],
                                         start=False, stop=(rr == 3))
                s2 = work.tile([P, N], f32, name="s2", tag="s2", bufs=2)
                nc.vector.tensor_add(s2[:], st[:], p2cT[:, pB * N:(pB + 1) * N])
                e = work.tile([P, N], bf16, name="e", tag="e", bufs=2)
                nc.scalar.activation(e[:], s2[:], Exp, scale=SCALE)
                for c in range(2):
                    nc.tensor.matmul(pb[:, 512 * c:512 * (c + 1)],
                                     va[jt][:, 66 * h:66 * h + 65],
                                     e[:, 512 * c:512 * (c + 1)],
                                     start=(jt == 0), stop=(jt == 7))

            # normalize: ctxT[hd] = pb[0:64] * (1/pb[64]) + bv
            rec = work.tile([1, N], bf16, name="rec", tag="rec", bufs=2)
            with nc.allow_low_precision(reason="bf16 softmax-recip broadcast"):
                nc.vector.reciprocal(rec[:], pb[64:65, :])
            rbc = psS.tile([P, N], f32, name="rbc", tag="st2")
            for c in range(2):
                nc.tensor.matmul(rbc[0:64, 512 * c:512 * (c + 1)], ones1[:, 0:64],
                                 rec[:, 512 * c:512 * (c + 1)],
                                 start=True, stop=True)
            rbs = work.tile([64, N], bf16, name="rbs", tag="rbs", bufs=2)
            nc.scalar.copy(rbs[:], rbc[0:64, :])
            nc.vector.tensor_mul(ctxT[hd, :], pb[0:64, :], rbs[:])


        wot_sb, _wot_free = tc.tile([P, HID], bf16, name="wot_sb")
        nc.sync.dma_start(wot_sb[:], wot[:])
        nc.scalar.activation(dume[:, 0:1], dume[:, 1:2], Sqrt)

        # ---- output dense partials over all rows -> ReduceScatter --------
        opart = dwork.tile((N, HID), bf16, name="opart", tag="opart")
        rs_out = dwork.tile((P, HID), bf16, name="rs_out", tag="rs_out")
        for it in range(8):
            osb = work.tile([P, HID], bf16, name="osb", tag="osb", bufs=2)
            po = psS.tile([P, N], f32, name="po", tag="st2")
            for c in range(2):
                nc.tensor.matmul(po[:, 512 * c:512 * (c + 1)],
                                 ctxT[:, P * it:P * (it + 1)],
                                 wot_sb[:, 512 * c:512 * (c + 1)],
                                 start=True, stop=True)
            if it % 2 == 0:
                nc.scalar.copy(osb[:], po[:])
            else:
                nc.vector.tensor_copy(osb[:], po[:])
            nc.sync.dma_start(opart[P * it:P * (it + 1), :], osb[:])
        nc.gpsimd.collective_compute(
            "ReduceScatter", ADD, replica_groups=[list(range(NCORES))],
            ins=[opart[:]], outs=[rs_out[:]])

        # ---- ln_g / ln_b broadcast via K=1 matmul ------------------------
        g_sb, _gf = tc.tile([P, HID], f32, name="g_sb")
        b_sb, _bf = tc.tile([P, HID], f32, name="b_sb")
        for half, dst in ((0, g_sb), (1, b_sb)):
            pg = psS.tile([P, N], f32, name="pg", tag="st2")
            for c in range(2):
                nc.tensor.matmul(pg[:, 512 * c:512 * (c + 1)], ones1[:],
                                 gl_bf[:, half * HID + 512 * c:half * HID + 512 * (c + 1)],
                                 start=True, stop=True)
            nc.vector.tensor_copy(dst[:], pg[:])

        # ---- residual + LayerNorm on own 128 rows ------------------------
        xr, _xrf = tc.tile([P, HID], bf16, name="xr")
        nc.sync.dma_start(xr[:], rs_out[:])
        x, _xf = tc.tile([P, HID], f32, name="x")
        nc.vector.tensor_add(x[:], xr[:], hsr_sb[:])

        stats = wt.tile([P, 2, 6], f32, name="stats", tag="stats")
        mv = wt.tile([P, 2], f32, name="mv", tag="mv")
        for s in range(2):
            nc.vector.bn_stats(stats[:, s, :], x[:, 512 * s:512 * (s + 1)])
        nc.vector.bn_aggr(mv[:], stats[:])
        epsb = wt.tile([P, 1], f32, name="epsb", tag="epsb")
        nc.vector.memset(epsb[:], EPS)
        std = wt.tile([P, 1], f32, name="std", tag="std")
        nc.scalar.activation(std[:], mv[:, 1:2], Sqrt, bias=epsb[:])
        rstd = wt.tile([P, 1], f32, name="rstd", tag="rstd")
        nc.vector.reciprocal(rstd[:], std[:])

        t1, _t1f = tc.tile([P, HID], f32, name="t1")
        nc.vector.scalar_tensor_tensor(t1[:], x[:], mv[:, 0:1], g_sb[:],
                                       op0=SUB, op1=MUL)
        yout, _yf = tc.tile([P, HID], f32, name="yout")
        for cc in range(2):
            sl = slice(512 * cc, 512 * (cc + 1))
            nc.vector.scalar_tensor_tensor(yout[:, sl], t1[:, sl], rstd[:],
                                           b_sb[:, sl], op0=MUL, op1=ADD)
            nc.sync.dma_start(out_t[:, sl], yout[:, sl])
        for f in (_yf, _t1f, _xf, _xrf, _bf, _gf, _wot_free):
            f()

    nc.compile()
    return nc, names


def _get_compiled():
    if "nc" not in _CACHE:
        nc, names = _build()
        _CACHE["nc"] = nc
        _CACHE["names"] = names
    return _CACHE["nc"], _CACHE["names"]


def _pack8(mat, width):
    # (1024, width) -> [128, 8*width]: out[p, kt*width+c] = mat[128*kt+p, c]
    return np.ascontiguousarray(
        mat.reshape(8, P, width).transpose(1, 0, 2).reshape(P, 8 * width))


def _prep_in_maps(inputs):
    import ml_dtypes

    bf = ml_dtypes.bfloat16
    hs = np.asarray(inputs["hidden_states"], np.float32)[0]      # (N, HID)
    rel = np.asarray(inputs["rel_embeddings"], np.float32)       # (2K, HID)
    from ml_dtypes import float8_e4m3fn as f8
    hst = _pack8(np.ascontiguousarray(hs.T), N).astype(bf)
    relT = _pack8(np.ascontiguousarray(rel.T), 2 * K).astype(f8)
    Wo = np.asarray(inputs["Wo"], np.float32)

    kinds = ["Wq", "Wk", "Wv", "Wpk", "Wpq"]
    bias_names = ["bq", "bk", "bv", "bpk", "bpq"]

    in_maps = []
    for r in range(NCORES):
        wpack = np.zeros((P, 5 * 8 * P), np.float32)
        for ki, kn in enumerate(kinds):
            w = np.asarray(inputs[kn], np.float32)
            wt = np.ascontiguousarray(w[DPC * r:DPC * (r + 1), :].T)  # (1024,128)
            wpack[:, ki * 8 * P:(ki + 1) * 8 * P] = _pack8(wt, P)
        wpos8 = np.zeros((P, 2 * 8 * P), np.float32)
        for ki, kn in enumerate(["Wpk", "Wpq"]):
            w = np.asarray(inputs[kn], np.float32)
            wtt = np.ascontiguousarray(w[DPC * r:DPC * (r + 1), :].T)
            wpos8[:, ki * 8 * P:(ki + 1) * 8 * P] = _pack8(wtt, P)
        bias8 = np.zeros((P, 8), np.float32)
        for bi, bn in enumerate(bias_names):
            bias8[:, bi] = np.asarray(inputs[bn], np.float32)[DPC * r:DPC * (r + 1)]
        glrow = np.concatenate([np.asarray(inputs["ln_g"], np.float32),
                                np.asarray(inputs["ln_b"], np.float32)])[None, :]
        hsr = (np.ascontiguousarray(hs[P * r:P * (r + 1), :])
               + np.asarray(inputs["bo"], np.float32)[None, :]
               + (np.asarray(inputs["bv"], np.float32)
                  @ Wo.T.astype(np.float32))[None, :])
        m = {
            "hst": hst,
            "relT": relT,
            "wpack": wpack.astype(bf),
            "wpos8": wpos8.astype(f8),
            "wot": np.ascontiguousarray(
                Wo[:, DPC * r:DPC * (r + 1)].T).astype(bf),
            "hsr": np.ascontiguousarray(hsr.astype(np.float32)),
            "bias8": bias8,
            "gl": np.ascontiguousarray(glrow),
        }
        in_maps.append(m)
    return in_maps


def run(inputs, trace=False):
    from concourse.bass_utils import run_bass_kernel_spmd

    nc, names = _get_compiled()
    logical = _prep_in_maps(inputs)
    in_maps = [{names[k]: v for k, v in m.items()} for m in logical]
    res = run_bass_kernel_spmd(nc, in_maps, list(range(NCORES)), trace=trace)
    outs = [res.results[r][names["out"]].astype(np.float32) for r in range(NCORES)]
    full = np.concatenate(outs, axis=0).reshape(1, N, HID)
    return full, res


def kernel(**inputs) -> np.ndarray:
    full, _ = run(inputs, trace=False)
    return full
